# revision 13
# baseline (speedup 1.0000x reference)
"""GRU free-run greedy decoder on 8 Trainium2 NeuronCores (data parallel), v4.

Precision scheme — "compensated f32r", 3 passes per h-dependent matmul:
TRN2 f32r matmuls round BOTH operands to 12 mantissa bits (RN-ties-even,
probed on HW) but run at bf16 rate (1 cycle/row at moving>=256) vs fp32's
4 cycles/row. Each h @ W accumulates in PSUM as:
    pass1 f32r: h   @ W    -> h12*W12   (PE's own operand rounding)
    pass2 f32r: hr  @ W    -> hr *W12   (hr = h - RN12(h), <=12 sig bits)
    pass3 fp16: (h*2^-6) @ (Wr*2^6) -> h*Wr   (Wr = W - RN12even(W))
dropping only hr*Wr ~ 2^-25; scheme error ~2^-21 relative. Exact numpy
replay (RN12even + fp16 quantization): 0 flipped tokens in all 65536
decode positions (flip threshold sits at ~2^-18: 1-2-pass schemes flip
100+ positions -> rel_err 0.06+, measured).

h12/hr come from one fp32 PE-transpose: h^T lands in PSUM, a dtype cast
produces h12T (f32r SBUF), one DVE sub produces hrT. Biases enter PSUM
as exact PE seeds: per-row tensors (Lc, b0n) as RN12 + residual f32r
pairs, partition-constant ones as a K=3 matmul against an exact 3-way
bf16 decomposition. Lc (latent @ W_lat + bias, constant across steps) is
folded on the host. The one-hot embedding runs as f32r + fp16-residual
passes (one-hot rows are exact in any dtype).
"""

import sys
import numpy as np

sys.path.insert(0, "/opt/trn_rl_repo")

P = 128          # partitions == per-core batch
H = 512          # hidden
V = 256          # vocab
LAT = 256        # latent dim
G3 = 3 * H       # 1536 gate width
T_FULL = 64
N_CORES = 8

_CACHE = {}


def build_program(T=T_FULL):
    import concourse.bass as bass
    import concourse.tile as tile
    from concourse import bacc, mybir
    from concourse.masks import make_identity

    f32 = mybir.dt.float32
    f32r = mybir.dt.float32r
    f16 = mybir.dt.float16
    bf16 = mybir.dt.bfloat16
    AF = mybir.ActivationFunctionType
    OP = mybir.AluOpType
    ts = bass.ts

    nc = bacc.Bacc(
        "TRN2", target_bir_lowering=False, debug=False,
        enable_asserts=False, num_devices=N_CORES,
    )

    # ---- DRAM I/O ----
    # f32r tensors feed f32r matmuls (walrus requires producer dtype match);
    # f16 tensors are the *2^6-scaled RN12 residuals for pass 3.
    wembT_d = nc.dram_tensor("wembT", [2, P, G3], f32r, kind="ExternalInput").ap()
    wembr_d = nc.dram_tensor("wembr", [2, P, G3], f16, kind="ExternalInput").ap()
    whh0T_d = nc.dram_tensor("whh0T", [4, P, G3], f32r, kind="ExternalInput").ap()
    wih1T_d = nc.dram_tensor("wih1T", [4, P, G3], f32r, kind="ExternalInput").ap()
    whh1T_d = nc.dram_tensor("whh1T", [4, P, G3], f32r, kind="ExternalInput").ap()
    whh0r_d = nc.dram_tensor("whh0r", [4, P, G3], f16, kind="ExternalInput").ap()
    wih1r_d = nc.dram_tensor("wih1r", [4, P, G3], f16, kind="ExternalInput").ap()
    whh1r_d = nc.dram_tensor("whh1r", [4, P, G3], f16, kind="ExternalInput").ap()
    wfcT_d = nc.dram_tensor("wfcT", [4, P, V], f32r, kind="ExternalInput").ap()
    wfcr_d = nc.dram_tensor("wfcr", [4, P, V], f16, kind="ExternalInput").ap()
    # per-row seed tensors: RN12 part + residual (both f32r-exact)
    lcrz12_d = nc.dram_tensor("lcrz12", [P, 2 * H], f32r, kind="ExternalInput").ap()
    lcrzr_d = nc.dram_tensor("lcrzr", [P, 2 * H], f32r, kind="ExternalInput").ap()
    b0n12_d = nc.dram_tensor("b0n12", [P, H], f32r, kind="ExternalInput").ap()
    b0nr_d = nc.dram_tensor("b0nr", [P, H], f32r, kind="ExternalInput").ap()
    b0hns_d = nc.dram_tensor("b0hns", [3, H], bf16, kind="ExternalInput").ap()
    # partition-constant biases: exact 3-way bf16 stacks [3, N]
    b1rzs_d = nc.dram_tensor("b1rzs", [3, 2 * H], bf16, kind="ExternalInput").ap()
    b1ns_d = nc.dram_tensor("b1ns", [3, 2 * H], bf16, kind="ExternalInput").ap()
    bfcs_d = nc.dram_tensor("bfcs", [3, V], bf16, kind="ExternalInput").ap()
    idn_d = nc.dram_tensor("idn", [P, P], f32r, kind="ExternalInput").ap()
    out_d = nc.dram_tensor("out", [P, T, V], f32, kind="ExternalOutput").ap()

    from contextlib import ExitStack
    with tile.TileContext(nc) as tc, ExitStack() as ctx:
        wt = ctx.enter_context(tc.tile_pool(name="wt", bufs=1))
        st = ctx.enter_context(tc.tile_pool(name="st", bufs=1))
        wk = ctx.enter_context(tc.tile_pool(name="wk", bufs=1))
        ps = ctx.enter_context(tc.tile_pool(name="ps", bufs=2, space="PSUM"))
        ps1 = ctx.enter_context(tc.tile_pool(name="ps1", bufs=1, space="PSUM"))

        # ---- persistent weights/biases in SBUF ----
        whh0T = wt.tile([P, 4, G3], f32r, tag="whh0T")
        wih1T = wt.tile([P, 4, G3], f32r, tag="wih1T")
        whh1T = wt.tile([P, 4, G3], f32r, tag="whh1T")
        whh0r = wt.tile([P, 4, G3], f16, tag="whh0r")
        wih1r = wt.tile([P, 4, G3], f16, tag="wih1r")
        whh1r = wt.tile([P, 4, G3], f16, tag="whh1r")
        wembT = wt.tile([P, 2, G3], f32r, tag="wembT")
        wembr = wt.tile([P, 2, G3], f16, tag="wembr")
        wfcT = wt.tile([P, 4, V], f32r, tag="wfcT")
        wfcr = wt.tile([P, 4, V], f16, tag="wfcr")

        # DMA order == first-use order. The tiny seed tensors go FIRST so
        # the t=0 seed matmuls don't sit behind ~17MB of weights (measured:
        # a 49us PE stall at program start with the old order).
        lcrz12 = wt.tile([P, 2 * H], f32r, tag="lcrz12")
        lcrzr = wt.tile([P, 2 * H], f32r, tag="lcrzr")
        b0n12 = wt.tile([P, H], f32r, tag="b0n12")
        b0nr = wt.tile([P, H], f32r, tag="b0nr")
        b0hns = wt.tile([3, H], bf16, tag="b0hns")
        b1rzs = wt.tile([3, 2 * H], bf16, tag="b1rzs")
        b1ns = wt.tile([3, 2 * H], bf16, tag="b1ns")
        bfcs = wt.tile([3, V], bf16, tag="bfcs")
        idn = wt.tile([P, P], f32r, tag="idn")
        nc.sync.dma_start(idn[:], idn_d[:])
        nc.sync.dma_start(lcrz12[:], lcrz12_d[:])
        nc.sync.dma_start(lcrzr[:], lcrzr_d[:])
        nc.sync.dma_start(b0n12[:], b0n12_d[:])
        nc.sync.dma_start(b0nr[:], b0nr_d[:])
        nc.sync.dma_start(b0hns[:], b0hns_d[:])
        nc.sync.dma_start(b1rzs[:], b1rzs_d[:])
        nc.sync.dma_start(b1ns[:], b1ns_d[:])
        nc.sync.dma_start(bfcs[:], bfcs_d[:])
        for kc in range(4):
            nc.sync.dma_start(wih1T[:, kc, :], wih1T_d[kc])
            nc.sync.dma_start(wih1r[:, kc, :], wih1r_d[kc])
        for kc in range(4):
            nc.sync.dma_start(whh0T[:, kc, :], whh0T_d[kc])
            nc.sync.dma_start(whh0r[:, kc, :], whh0r_d[kc])
        for kc in range(4):
            nc.sync.dma_start(wfcT[:, kc, :], wfcT_d[kc])
            nc.sync.dma_start(wfcr[:, kc, :], wfcr_d[kc])
        for kc in range(4):
            nc.sync.dma_start(whh1T[:, kc, :], whh1T_d[kc])
            nc.sync.dma_start(whh1r[:, kc, :], whh1r_d[kc])
        for kc in range(2):
            nc.sync.dma_start(wembT[:, kc, :], wembT_d[kc])
            nc.sync.dma_start(wembr[:, kc, :], wembr_d[kc])

        ones3 = wt.tile([3, P], bf16, tag="ones3")
        nc.gpsimd.memset(ones3[:], 1.0)
        zer = wt.tile([P, H], bf16, tag="zer")
        nc.gpsimd.memset(zer[:], 0.0)
        identb = wt.tile([P, P], bf16, tag="identb")
        make_identity(nc, identb[:])
        ident = wt.tile([P, P], f32, tag="ident")
        make_identity(nc, ident[:])

        # ---- persistent state ----
        h0 = st.tile([P, H], f32, tag="h0")
        h1 = st.tile([P, H], f32, tag="h1")
        h0T = st.tile([P, 4, P], f32r, tag="h0T")     # RN12-ish(h0)^T
        h0rT = st.tile([P, 4, P], f32r, tag="h0rT")   # residual^T
        h0mT = st.tile([P, 4, P], f16, tag="h0mT")    # fp16(h0^T * 2^-6)
        h1T = st.tile([P, 4, P], f32r, tag="h1T")
        h1rT = st.tile([P, 4, P], f32r, tag="h1rT")
        h1mT = st.tile([P, 4, P], f16, tag="h1mT")
        ohT = st.tile([P, 2, P], f32r, tag="ohT")
        ohTm = st.tile([P, 2, P], f16, tag="ohTm")    # one-hot^T * 2^-6
        for tl in (h0, h1):
            nc.gpsimd.memset(tl[:], 0.0)
        nc.gpsimd.memset(h0T[:, :, :].bitcast(f32), 0.0)
        nc.gpsimd.memset(h0rT[:, :, :].bitcast(f32), 0.0)
        nc.gpsimd.memset(h0mT[:, :, :], 0.0)
        nc.gpsimd.memset(h1T[:, :, :].bitcast(f32), 0.0)
        nc.gpsimd.memset(h1rT[:, :, :].bitcast(f32), 0.0)
        nc.gpsimd.memset(h1mT[:, :, :], 0.0)
        nc.gpsimd.memset(ohT[:, :, :].bitcast(f32), 0.0)
        nc.gpsimd.memset(ohTm[:, :, :], 0.0)

        def seed_rowwise(dest, s12, sres, n):
            """Exact per-row seed: RN12 part + residual, both f32r passes."""
            for ci in range(0, n, 512):
                w = min(512, n - ci)
                nc.tensor.matmul(dest[:, ci:ci + w], idn[:],
                                 s12[:, ci:ci + w], start=True, stop=False)
                nc.tensor.matmul(dest[:, ci:ci + w], idn[:],
                                 sres[:, ci:ci + w], start=False, stop=False)

        def seed_const(dest, stk, n, stop=False):
            """Exact partition-constant seed: ones3 @ 3-way bf16 stack."""
            for ci in range(0, n, 512):
                w = min(512, n - ci)
                nc.tensor.matmul(dest[:, ci:ci + w], ones3[:],
                                 stk[:, ci:ci + w], start=True, stop=stop)

        def mm3(dest, hT, hrT, hmT, w, wr, col, width, stop=False):
            """3-pass compensated-f32r accumulation of h @ W[:, col:col+width]."""
            for kc in range(4):
                nc.tensor.matmul(dest, hT[:, kc, :], w[:, kc, col:col + width],
                                 start=False, stop=False)
            for kc in range(4):
                nc.tensor.matmul(dest, hrT[:, kc, :], w[:, kc, col:col + width],
                                 start=False, stop=False)
            for kc in range(4):
                nc.tensor.matmul(dest, hmT[:, kc, :], wr[:, kc, col:col + width],
                                 start=False, stop=(stop and kc == 3))

        def gru_gates(grz, gihn, h, tag):
            """gates + state update for one layer; h updated in place."""
            rr = wk.tile([P, H], f32, tag="rr", name=f"rr{tag}")
            nc.scalar.activation(rr[:], grz[:, 0:512], AF.Sigmoid)
            zz = wk.tile([P, H], f32, tag="zz", name=f"zz{tag}")
            nc.scalar.activation(zz[:], grz[:, 512:1024], AF.Sigmoid)
            rhn = wk.tile([P, H], f32, tag="rhn", name=f"rhn{tag}")
            nc.vector.tensor_mul(rhn[:], rr[:], gihn[:, 512:1024])
            npre = wk.tile([P, H], f32, tag="npre", name=f"npre{tag}")
            nc.vector.tensor_add(npre[:], gihn[:, 0:512], rhn[:])
            nn = wk.tile([P, H], f32, tag="nn", name=f"nn{tag}")
            nc.scalar.activation(nn[:], npre[:], AF.Tanh)
            # h' = n + z*(h - n)
            dd = wk.tile([P, H], f32, tag="dd", name=f"dd{tag}")
            nc.gpsimd.tensor_sub(dd[:], h[:], nn[:])
            zd = wk.tile([P, H], f32, tag="zd", name=f"zd{tag}")
            nc.vector.tensor_mul(zd[:], zz[:], dd[:])
            nc.gpsimd.tensor_add(h[:], nn[:], zd[:])

        def transpose_split(h, hT, hrT, hmT, tail):
            """h [P,512] -> fp32 h^T in tail psum [0:512]; then
            hT = f32r cast (rounds), hrT = psum - hT, hmT = fp16(psum*2^-6)."""
            for kc in range(4):
                nc.tensor.transpose(tail[:, ts(kc, P)], h[:, ts(kc, P)], ident[:])
            tsl = tail[:, 0:512]
            nc.vector.tensor_copy(hT[:, :, :].rearrange("p a b -> p (a b)"), tsl)
            nc.vector.tensor_sub(hrT[:, :, :].rearrange("p a b -> p (a b)"),
                                 tsl, hT[:, :, :].rearrange("p a b -> p (a b)").bitcast(f32))
            nc.scalar.mul(hmT[:, :, :].rearrange("p a b -> p (a b)"), tsl, 2.0 ** -6)

        def argmax_tail(t, tail, lg):
            """argmax(lg, psum) -> one-hot -> DMA + ohT/ohTm (tail 256:512)."""
            mx = wk.tile([P, 1], f32, tag="mx", name=f"mx_{t}")
            nc.vector.reduce_max(mx[:], lg, axis=mybir.AxisListType.X)
            oh = wk.tile([P, V], f32, tag="oh", name=f"oh_{t}")
            nc.vector.tensor_scalar(oh[:], lg, mx[:, 0:1], None,
                                    op0=OP.is_equal)
            nc.sync.dma_start(out_d[:, t, :], oh[:])
            if tail is not None:
                for v in range(2):
                    nc.tensor.transpose(tail[:, 256 + v * P:256 + (v + 1) * P],
                                        oh[:, ts(v, P)], ident[:])
                tsl = tail[:, 256:512]
                nc.scalar.copy(
                    ohT[:, :, :].rearrange("p a b -> p (a b)"), tsl)
                nc.scalar.mul(
                    ohTm[:, :, :].rearrange("p a b -> p (a b)"), tsl, 2.0 ** -6)

        # ---- the T decode steps ----
        prev_tail = None
        prev_lg = None
        ng0rz = ng1rz = None
        for t in range(T):
            if t == 0:
                g0rz = ps.tile([P, 1024], f32, tag="rz", name="g0rz_0")
                seed_rowwise(g0rz, lcrz12, lcrzr, 1024)
                g1rz = ps.tile([P, 1024], f32, tag="rz", name="g1rz_0")
                seed_const(g1rz, b1rzs, 1024)
            else:
                g0rz, g1rz = ng0rz, ng1rz

            # -- step t-1 tail first: argmax -> one-hot -> ohT. gh1_rz
            # pass1 fills the PE while the DVE computes the argmax (it only
            # touches t-1 state). g0ihn was seeded + gh0_n-filled at the end
            # of step t-1, so the l0 ihn group closes right after emb_n. --
            if t == 0:
                g0ihn = ps1.tile([P, 1024], f32, tag="ihn", name="g0ihn_0")
                seed_rowwise(g0ihn[:, 0:512], b0n12, b0nr, 512)
                seed_const(g0ihn[:, 512:1024], b0hns, 512)
            else:
                g0ihn = ng0ihn
            if t > 0:
                for j in (0, 1):  # gh1_rz pass1: boundary filler
                    for kc in range(4):
                        nc.tensor.matmul(g1rz[:, ts(j, 512)], h1T[:, kc, :],
                                         whh1T[:, kc, ts(j, 512)],
                                         start=False, stop=False)
                argmax_tail(t - 1, prev_tail, prev_lg)

            # -- emb closes the l0 groups asap: rz regions first (shortest
            # path to sig_r), wembT passes before the wembr ones so the PE
            # never waits on the ohTm scale-copy --
            if t > 0:
                for j in (0, 1):
                    for v in range(2):
                        nc.tensor.matmul(g0rz[:, ts(j, 512)], ohT[:, v, :],
                                         wembT[:, v, ts(j, 512)],
                                         start=False, stop=False)
                for j in (0, 1):
                    for v in range(2):
                        nc.tensor.matmul(g0rz[:, ts(j, 512)], ohTm[:, v, :],
                                         wembr[:, v, ts(j, 512)],
                                         start=False, stop=(v == 1))
                for v in range(2):
                    nc.tensor.matmul(g0ihn[:, 0:512], ohT[:, v, :],
                                     wembT[:, v, 1024:1536],
                                     start=False, stop=False)
                for v in range(2):
                    nc.tensor.matmul(g0ihn[:, 0:512], ohTm[:, v, :],
                                     wembr[:, v, 1024:1536],
                                     start=False, stop=(v == 1))
            else:
                for j in (0, 1):
                    nc.tensor.matmul(g0rz[:, ts(j, 512)], identb[:], zer[:],
                                     start=False, stop=True)
                nc.tensor.matmul(g0ihn[:, 0:512], identb[:], zer[:],
                                 start=False, stop=True)

            if t > 0:
                # gh1 rz residual passes; pass1 ran at the step boundary.
                # Overlaps the l0 gate chain.
                for j in (0, 1):
                    for kc in range(4):
                        nc.tensor.matmul(g1rz[:, ts(j, 512)], h1rT[:, kc, :],
                                         whh1T[:, kc, ts(j, 512)],
                                         start=False, stop=False)
                    for kc in range(4):
                        nc.tensor.matmul(g1rz[:, ts(j, 512)], h1mT[:, kc, :],
                                         whh1r[:, kc, ts(j, 512)],
                                         start=False, stop=False)
            else:
                nc.tensor.matmul(g0ihn[:, 512:1024], identb[:], zer[:],
                                 start=False, stop=True)

            # -- layer0 gates -> h0 --
            gru_gates(g0rz, g0ihn, h0, f"0_{t}")

            # -- gh1_n: runnable while DVE computes the l0 gates --
            g1ihn = ps1.tile([P, 1024], f32, tag="ihn", name=f"g1ihn_{t}")
            seed_const(g1ihn, b1ns, 1024)
            if t > 0:
                mm3(g1ihn[:, 512:1024], h1T, h1rT, h1mT, whh1T, whh1r,
                    1024, 512, stop=True)
            else:
                nc.tensor.matmul(g1ihn[:, 512:1024], identb[:], zer[:],
                                 start=False, stop=True)

            # -- h0'^T -> h0T / h0rT / h0mT --
            tail = ps1.tile([P, 1024], f32, tag="tail", name=f"tail_{t}")
            transpose_split(h0, h0T, h0rT, h0mT, tail)

            # -- gi1 (= h0' @ Wih1T), rz first then i_n --
            for j in (0, 1):
                mm3(g1rz[:, ts(j, 512)], h0T, h0rT, h0mT, wih1T, wih1r,
                    j * 512, 512, stop=True)
            mm3(g1ihn[:, 0:512], h0T, h0rT, h0mT, wih1T, wih1r, 1024, 512,
                stop=True)

            # -- next step's hh0_rz: h0T fresh; PE chews during l1 gates --
            if t + 1 < T:
                ng0rz = ps.tile([P, 1024], f32, tag="rz", name=f"g0rz_{t+1}")
                seed_rowwise(ng0rz, lcrz12, lcrzr, 1024)
                for j in range(2):
                    mm3(ng0rz[:, ts(j, 512)], h0T, h0rT, h0mT, whh0T, whh0r,
                        j * 512, 512)
            else:
                # t == T-1: no prefetch work exists, so the in-order PE queue
                # would run dry while the last l1 gate chain completes
                # (measured 5.9us drain stall). Feed it self-contained dummy
                # groups into tail[256:512] (dead after argmax(T-2)'s ohT
                # copy; overwritten-by-WAR before the h1 transposes).
                for _ in range(22):
                    nc.tensor.matmul(tail[:, 256:512], identb[:],
                                     zer[:, 0:256], start=True, stop=True)

            # -- layer1 gates -> h1 --
            gru_gates(g1rz, g1ihn, h1, f"1_{t}")

            # -- next step's g1rz seed --
            if t + 1 < T:
                ng1rz = ps.tile([P, 1024], f32, tag="rz", name=f"g1rz_{t+1}")
                seed_const(ng1rz, b1rzs, 1024)

            # -- h1'^T --
            transpose_split(h1, h1T, h1rT, h1mT, tail)

            # -- fc logits (+bias seed) -> tail cols [0:256] --
            seed_const(tail[:, 0:256], bfcs, 256)
            for kc in range(4):
                nc.tensor.matmul(tail[:, 0:256], h1T[:, kc, :],
                                 wfcT[:, kc, :], start=False, stop=False)
            for kc in range(4):
                nc.tensor.matmul(tail[:, 0:256], h1rT[:, kc, :],
                                 wfcT[:, kc, :], start=False, stop=False)
            for kc in range(4):
                nc.tensor.matmul(tail[:, 0:256], h1mT[:, kc, :],
                                 wfcr[:, kc, :], start=False, stop=(kc == 3))
            prev_tail, prev_lg = tail, tail[:, 0:256]

            # -- prefetch step t+1's l0 ihn group: seeds + gh0_n (all only
            # need h0T of this step; the ihn slot frees once l1's rhn/npre
            # read g1ihn). Runs while the DVE does the t-boundary argmax,
            # and lets t+1's ihn group close right after emb_n. --
            if t + 1 < T:
                ng0ihn = ps1.tile([P, 1024], f32, tag="ihn",
                                  name=f"g0ihn_{t+1}")
                seed_rowwise(ng0ihn[:, 0:512], b0n12, b0nr, 512)
                seed_const(ng0ihn[:, 512:1024], b0hns, 512)
                mm3(ng0ihn[:, 512:1024], h0T, h0rT, h0mT, whh0T, whh0r,
                    1024, 512, stop=True)

        argmax_tail(T - 1, None, prev_lg)

    nc.compile()
    return nc


def _rn12_even(a):
    """RN-ties-even to 12 mantissa bits (11 explicit) — matches TRN2 f32r."""
    u = np.ascontiguousarray(a.astype(np.float32)).view(np.uint32).copy()
    sign = u & np.uint32(0x80000000)
    mag = u & np.uint32(0x7FFFFFFF)
    mag = (mag + np.uint32(0x7FF) + ((mag >> np.uint32(12)) & np.uint32(1))) \
        & np.uint32(0xFFFFF000)
    return (sign | mag).view(np.float32)


def _split3_bf16(a):
    """EXACT 3-way bf16 split: a == hi + mid + lo in fp32 (any add order)."""
    import ml_dtypes
    f4, bf = np.float32, ml_dtypes.bfloat16
    a = a.astype(f4)
    hi = a.astype(bf)
    r = a - hi.astype(f4)
    mid = r.astype(bf)
    lo = (r - mid.astype(f4)).astype(bf)
    return np.ascontiguousarray(np.stack([hi, mid, lo]))


def prep_host_inputs(latent_vec, w_ih0, w_hh0, b_ih0, b_hh0,
                     w_ih_r, w_hh_r, b_ih_r, b_hh_r, w_fc, b_fc):
    """Host prep: transposes/reshapes, RN12 weight splits, Lc fold."""
    f4 = np.float32
    f2 = np.float16

    def wsplit(wT, kchunks):
        w = np.ascontiguousarray(wT.astype(f4)).reshape(kchunks, P, -1)
        wr = np.ascontiguousarray(((w - _rn12_even(w)) * 64.0).astype(f2))
        return w, wr

    wembT, wembr = wsplit(w_ih0[:, LAT:].T, 2)
    whh0T, whh0r = wsplit(w_hh0.T, 4)
    wih1T, wih1r = wsplit(w_ih_r[0].T, 4)
    whh1T, whh1r = wsplit(w_hh_r[0].T, 4)
    wfcT, wfcr = wsplit(w_fc.T, 4)

    # Lc = latent @ W_lat^T + b_ih0 (+ b_hh0 on the rz part): constant
    # across steps; computed here once in fp32 (pure input transform).
    Lc = latent_vec.astype(f4) @ w_ih0[:, :LAT].astype(f4).T + b_ih0.astype(f4)
    Lc[:, :1024] += b_hh0[:1024].astype(f4)

    b0hns = _split3_bf16(b_hh0[1024:])
    b1rzs = _split3_bf16(b_ih_r[0][:1024] + b_hh_r[0][:1024])
    b1ns = _split3_bf16(
        np.concatenate([b_ih_r[0][1024:], b_hh_r[0][1024:]]).astype(f4))
    bfcs = _split3_bf16(b_fc)
    idn = np.ascontiguousarray(np.eye(P, dtype=f4))

    common = dict(
        wembT=wembT, wembr=wembr,
        whh0T=whh0T, wih1T=wih1T, whh1T=whh1T,
        whh0r=whh0r, wih1r=wih1r, whh1r=whh1r,
        wfcT=wfcT, wfcr=wfcr,
        b0hns=b0hns, b1rzs=b1rzs, b1ns=b1ns, bfcs=bfcs, idn=idn,
    )
    in_maps = []
    for c in range(N_CORES):
        m = dict(common)
        lc_c = Lc[c * P:(c + 1) * P]
        lcrz = np.ascontiguousarray(lc_c[:, :1024])
        b0n = np.ascontiguousarray(lc_c[:, 1024:])  # lat_n + b_ih0_n (per-row)
        m["lcrz12"] = _rn12_even(lcrz)
        m["lcrzr"] = np.ascontiguousarray(lcrz - m["lcrz12"])
        m["b0n12"] = _rn12_even(b0n)
        m["b0nr"] = np.ascontiguousarray(b0n - m["b0n12"])
        in_maps.append(m)
    return in_maps


def kernel(**inputs):
    from concourse import bass_utils

    key = ("prog", T_FULL)
    if key not in _CACHE:
        _CACHE[key] = build_program(T_FULL)
    nc = _CACHE[key]

    in_maps = prep_host_inputs(
        np.asarray(inputs["latent_vec"]), np.asarray(inputs["w_ih0"]),
        np.asarray(inputs["w_hh0"]), np.asarray(inputs["b_ih0"]),
        np.asarray(inputs["b_hh0"]), np.asarray(inputs["w_ih_r"]),
        np.asarray(inputs["w_hh_r"]), np.asarray(inputs["b_ih_r"]),
        np.asarray(inputs["b_hh_r"]), np.asarray(inputs["w_fc"]),
        np.asarray(inputs["b_fc"]))

    res = bass_utils.run_bass_kernel_spmd(nc, in_maps, list(range(N_CORES)))
    out = np.concatenate([res.results[c]["out"] for c in range(N_CORES)], axis=0)
    return out.astype(np.float32)
